# revision 20
# baseline (speedup 1.0000x reference)
"""Trainium2 Bass kernel for a dense transformer block (pre-LN, MHA + GELU MLP).

Sharding: data-parallel over batch — B=8 batch elements map 1:1 onto the 8
NeuronCores; no collectives. Each core runs an identical SPMD program on its
own [1024, 768] slice.

Per-core dataflow (P=128 partitions):
  x normal [tok, d] --LN1 stats (bn_stats, DVE)--> xc=(x-mu)*rstd bf16
    (GpSimd) --PE transpose--> xhatT [d, tok] bf16.
  QKV: qT/kT = W.T @ xhatT (weights stationary, transposed out, bias fused in
    the PSUM->SBUF copy); v = xhatT.T @ Wv (normal layout) packed [tok,12,65]
    with a ones column per head so the ctx matmul's 65th row is the softmax
    row-sum.
  Attention per head-pair hp (head 2hp on partitions 0-63, 2hp+1 on 64-127):
    scoresT[j,i] = khT.T@qhT (K=64) for both heads into one 2-bank PSUM tile;
    one ACT Exp call per j covers both heads (no max subtraction: |s|<9 is
    fp32-safe); ctxU^T+rowsum = [v|1].T@expT (M=65). Normalize: exact DVE
    reciprocal of row 64, broadcast via DRAM bounce (step-0 partition AP),
    fused multiply into the bf16 copy; head 2hp+1's rows reach partitions
    64-127 of ctxT via a small SBUF->SBUF DMA (engines cannot shift
    partitions; DMA can).
  Wo: attn = ctxT.T @ Wo (normal out) + x residual fused; += bo' on GpSimd.
  LN2 like LN1 -> x2hatT; fc1 = W1.T @ x2hatT with bias+GELU fused on ACT;
  fc2 = gT.T @ W2 (normal out) + x2 residual fused -> out.
  The Wo/LN2/fc1 work for token chunk c is emitted right after attention
  chunk c so its PE work fills the ACT-bound attention phase.

Host-side folds (exact algebra): Wq' = diag(g1)Wq/8, bq' = (b1@Wq+bq)/8;
Wk'/bk' same unscaled; Wv' without bias (bo' = bo + (b1@Wv+bv)@Wo);
W1' = diag(g2)W1, b1' = b2ln@W1+b1. Weights cast to bf16 on host; all
accumulation fp32 on PE.
"""

from contextlib import ExitStack

import numpy as np
import ml_dtypes

import concourse.bass as bass
import concourse.mybir as mybir
from concourse import bacc
from concourse.tile import TileContext
from concourse.masks import make_identity
from concourse.bass_utils import run_bass_kernel_spmd

f32 = mybir.dt.float32
bf16 = mybir.dt.bfloat16
AF = mybir.ActivationFunctionType
ALU = mybir.AluOpType
ts = bass.ts

B = 8
N = 1024
D = 768
H = 12
DH = 64
FF = 3072
EPS = 1e-6
P = 128
NT = N // P    # 8 token tiles
DT = D // P    # 6 d tiles
FT = FF // P   # 24 ff tiles
CW = 512       # free-dim chunk (one PSUM bank of fp32)
NC_CHUNKS = N // CW  # 2
NCORES = 8

_PROGRAM = None
_TAPS = frozenset()
_tap_handles = {}
_W1TILES = []


def _tap(nc, name, aps):
    if name not in _TAPS:
        return
    shape = [len(aps)] + list(aps[0].shape)
    dt = aps[0].dtype
    h = nc.declare_dram_parameter(f"dbg_{name}", shape, dt, True)
    _tap_handles[f"dbg_{name}"] = shape
    for i, ap in enumerate(aps):
        nc.sync.dma_start(out=h[i], in_=ap)


def _bcast_ap(ap_row, parts):
    """AP reading one (DRAM) row broadcast across `parts` partitions."""
    return bass.AP(tensor=ap_row.tensor, offset=ap_row.offset,
                   ap=[[0, parts]] + list(ap_row.ap[1:]))


def _build_program():
    nc = bacc.Bacc("TRN2", target_bir_lowering=False, debug=False,
                   num_devices=NCORES)

    xd = nc.declare_dram_parameter("x", [N, D], f32, False)
    wqd = nc.declare_dram_parameter("wq", [D, D], bf16, False)
    wkd = nc.declare_dram_parameter("wk", [D, D], bf16, False)
    wvd = nc.declare_dram_parameter("wv", [D, D], bf16, False)
    wod = nc.declare_dram_parameter("wo", [D, D], bf16, False)
    w1d = nc.declare_dram_parameter("w1", [D, FF], bf16, False)
    w2d = nc.declare_dram_parameter("w2", [FF, D], bf16, False)
    bqd = nc.declare_dram_parameter("bq", [P, DT], f32, False)
    bkd = nc.declare_dram_parameter("bk", [P, DT], f32, False)
    b1d = nc.declare_dram_parameter("b1", [P, FT], f32, False)
    bobd = nc.declare_dram_parameter("bob", [P, D], f32, False)
    b2bd = nc.declare_dram_parameter("b2b", [P, D], f32, False)
    outd = nc.declare_dram_parameter("out", [N, D], f32, True)

    with TileContext(nc) as tc:
        _emit_body(nc, tc, xd, wqd, wkd, wvd, wod, w1d, w2d,
                   bqd, bkd, b1d, bobd, b2bd, outd)
    nc.compile()
    return nc


def _ln_tile(nc, pools, x_tile, xhatT, magic_t, ident, t, tag):
    """LN stats (DVE) + Newton rsqrt (GpSimd, keeps ACT free of table
    swaps) + center/scale (DVE, bf16) + PE transposes for one token tile;
    writes the t-th column block of each xhatT[j]."""
    ln_pool, xc_pool, ps_main = pools
    stats = ln_pool.tile([P, 3, 6], f32, tag=f"{tag}stats", name=f"{tag}st")
    for s3 in range(3):
        nc.vector.bn_stats(out=stats[:, s3, :],
                           in_=x_tile[:, s3 * 256:(s3 + 1) * 256])
    mv = ln_pool.tile([P, 2], f32, tag=f"{tag}mv", name=f"{tag}mv")
    nc.vector.bn_aggr(out=mv, in_=stats)
    # rstd = rsqrt(var+eps) via bit-trick seed + 2 Newton steps on GpSimd
    # ([P,1] tensors): avoids ACT Sqrt, whose table set would thrash with
    # the attention Exp set. Seed rel-err ~3.4e-2 -> ~4e-6 after 2 steps.
    v = ln_pool.tile([P, 1], f32, tag=f"{tag}v", name=f"{tag}v")
    nc.vector.tensor_scalar_add(out=v, in0=mv[:, 1:2], scalar1=EPS)
    y = ln_pool.tile([P, 1], f32, tag=f"{tag}y", name=f"{tag}y")
    hb = ln_pool.tile([P, 1], mybir.dt.uint32, tag=f"{tag}hb",
                      name=f"{tag}hb")
    nc.vector.tensor_scalar(out=hb, in0=v.bitcast(mybir.dt.uint32),
                            scalar1=1, scalar2=None,
                            op0=ALU.logical_shift_right)
    nc.vector.scalar_tensor_tensor(out=y.bitcast(mybir.dt.uint32),
                                   in0=magic_t, scalar=0,
                                   in1=hb, op0=ALU.add, op1=ALU.subtract)
    tN = ln_pool.tile([P, 1], f32, tag=f"{tag}tN", name=f"{tag}tN")
    for _ in range(2):
        nc.vector.tensor_mul(out=tN, in0=y, in1=y)
        nc.vector.tensor_mul(out=tN, in0=tN, in1=v)
        nc.vector.tensor_scalar(out=tN, in0=tN, scalar1=-0.5, scalar2=1.5,
                                op0=ALU.mult, op1=ALU.add)
        nc.vector.tensor_mul(out=y, in0=y, in1=tN)
    xc = xc_pool.tile([P, D], bf16, tag=f"{tag}xc", name=f"{tag}xc")
    nc.vector.tensor_scalar(out=xc, in0=x_tile, scalar1=mv[:, 0:1],
                            scalar2=y, op0=ALU.subtract, op1=ALU.mult)
    for j in range(DT):
        tp = ps_main.tile([P, P], bf16, tag="mm", bufs=2, name=f"{tag}tr")
        nc.tensor.transpose(tp, xc[:, ts(j, P)], ident)
        if (t * DT + j) % 2 == 0:
            nc.vector.tensor_copy(out=xhatT[j][:, ts(t, P)], in_=tp)
        else:
            nc.scalar.copy(out=xhatT[j][:, ts(t, P)], in_=tp)


def _emit_body(nc, tc, xd, wqd, wkd, wvd, wod, w1d, w2d,
               bqd, bkd, b1d, bobd, b2bd, outd):
    class Pools:
        """Manual open/close so SBUF/PSUM lifetimes match phase needs."""

        def __init__(self):
            self._cms = {}

        def open(self, name, **kw):
            cm = tc.tile_pool(name=name, **kw)
            pool = cm.__enter__()
            self._cms[name] = cm
            return pool

        def close(self, *names):
            for n in names:
                self._cms.pop(n).__exit__(None, None, None)

        def close_all(self):
            for n in reversed(list(self._cms)):
                self.close(n)

    pl = Pools()
    try:
        _emit_phases(nc, tc, pl, xd, wqd, wkd, wvd, wod, w1d, w2d,
                     bqd, bkd, b1d, bobd, b2bd, outd)
    finally:
        pl.close_all()


def _emit_phases(nc, tc, pl, xd, wqd, wkd, wvd, wod, w1d, w2d,
                 bqd, bkd, b1d, bobd, b2bd, outd):
    constp = pl.open("const", bufs=1)
    persist = pl.open("persist", bufs=1)
    ident = constp.tile([P, P], bf16, name="ident")
    make_identity(nc, ident)
    magic_t = constp.tile([P, 1], mybir.dt.uint32, name="magic")
    nc.vector.memset(magic_t, 0x5f3759df)
    bq_sb = constp.tile([P, DT], f32, name="bqs")
    nc.sync.dma_start(out=bq_sb, in_=bqd[:, :])
    bk_sb = constp.tile([P, DT], f32, name="bks")
    nc.sync.dma_start(out=bk_sb, in_=bkd[:, :])
    b1_sb = constp.tile([P, FT], f32, name="b1s")
    nc.sync.dma_start(out=b1_sb, in_=b1d[:, :])
    bo_b = constp.tile([P, D], f32, name="bob")
    nc.sync.dma_start(out=bo_b, in_=bobd[:, :])
    b2_b = constp.tile([P, D], f32, name="b2b")
    nc.sync.dma_start(out=b2_b, in_=b2bd[:, :])

    x2_sb = [persist.tile([P, D], f32, tag=f"x2_{t}", name=f"x2_{t}")
             for t in range(NT)]

    ln_pool = pl.open("ln", bufs=2)
    xc_pool = pl.open("xc", bufs=2)
    # One PSUM pool for the whole body; static bank budget (8):
    #   mm: 2 (transposes, qkv/Wo/fc groups)  s: 2x2 (scores + kv groups)
    #   c0/c1: 1+1 (ctx+rowsum accumulators)
    ps_main = pl.open("ps_main", bufs=1, space="PSUM")
    lnpools = (ln_pool, xc_pool, ps_main)

    # Long-lived pools first (stack order: deepest closes last).
    gp = pl.open("gT", bufs=1)
    x2hatp = pl.open("x2hatT", bufs=1)
    qkvp = pl.open("qkv", bufs=1)

    # ---------- Phase 1: LN1 + transpose (x tiles rotate) ----------
    xhatp = pl.open("xhatT", bufs=1)
    xln = pl.open("xln", bufs=3)
    xhatT = [xhatp.tile([P, N], bf16, tag=f"xh{j}", name=f"xh{j}")
             for j in range(DT)]
    for t in range(NT):
        xt = xln.tile([P, D], f32, tag="xln", name="xln")
        nc.sync.dma_start(out=xt, in_=xd[ts(t, P), :])
        _ln_tile(nc, lnpools, xt, xhatT, magic_t, ident, t, "l1")

    # ---------- Phase 2: QKV projections ----------
    qT = [qkvp.tile([P, N], bf16, tag=f"q{m}", name=f"q{m}")
          for m in range(DT)]
    kT = [qkvp.tile([P, N], bf16, tag=f"k{m}", name=f"k{m}")
          for m in range(DT)]
    v3 = [qkvp.tile([P, H, DH + 1], bf16, tag=f"v{t}", name=f"v{t}")
          for t in range(NT)]
    wp = pl.open("wqkv", bufs=1)
    wq_sb = [wp.tile([P, D], bf16, tag=f"wq{j}", name=f"wq{j}")
             for j in range(DT)]
    wk_sb = [wp.tile([P, D], bf16, tag=f"wk{j}", name=f"wk{j}")
             for j in range(DT)]
    wv_sb = [wp.tile([P, D], bf16, tag=f"wv{j}", name=f"wv{j}")
             for j in range(DT)]
    for j in range(DT):
        nc.sync.dma_start(out=wq_sb[j], in_=wqd[ts(j, P), :])
        nc.sync.dma_start(out=wk_sb[j], in_=wkd[ts(j, P), :])
        nc.sync.dma_start(out=wv_sb[j], in_=wvd[ts(j, P), :])
    for m in range(DT):
        for c in range(NC_CHUNKS):
            ps = ps_main.tile([P, CW], f32, tag="mm", bufs=2, name="qps")
            for j in range(DT):
                nc.tensor.matmul(ps, wq_sb[j][:, ts(m, P)],
                                 xhatT[j][:, ts(c, CW)],
                                 start=(j == 0), stop=(j == DT - 1))
            nc.vector.tensor_scalar_add(out=qT[m][:, ts(c, CW)], in0=ps,
                                        scalar1=bq_sb[:, m:m + 1])
            ps = ps_main.tile([P, 2, CW], f32, tag="s", bufs=2,
                              name="kps")[:, 0, :]
            for j in range(DT):
                nc.tensor.matmul(ps, wk_sb[j][:, ts(m, P)],
                                 xhatT[j][:, ts(c, CW)],
                                 start=(j == 0), stop=(j == DT - 1))
            nc.scalar.activation(out=kT[m][:, ts(c, CW)], in_=ps,
                                 func=AF.Identity, bias=bk_sb[:, m:m + 1])
    for t in range(NT):
        nc.vector.memset(v3[t][:, :, DH:DH + 1], 1.0)
        for lo, w in ((0, 512), (512, 256)):
            ps = ps_main.tile([P, 2, CW], f32, tag="s", bufs=2,
                              name="vps")[:, 0, :]
            for j in range(DT):
                nc.tensor.matmul(ps[:, 0:w], xhatT[j][:, ts(t, P)],
                                 wv_sb[j][:, lo:lo + w],
                                 start=(j == 0), stop=(j == DT - 1))
            h0, nh = lo // DH, w // DH
            nc.vector.tensor_copy(
                out=v3[t][:, h0:h0 + nh, 0:DH],
                in_=ps[:, 0:w].rearrange("p (h d) -> p h d", d=DH))
    _tap(nc, "xh", xhatT)
    _tap(nc, "q", qT)
    _tap(nc, "k", kT)
    _tap(nc, "v", v3)
    pl.close("wqkv", "xln", "xhatT")

    # ---------- Phases 3-5, interleaved by token chunk c ----------
    ctxT = [qkvp.tile([P, N], bf16, tag=f"ctx{m}", name=f"ctx{m}")
            for m in range(DT)]
    x2hatT = [x2hatp.tile([P, N], bf16, tag=f"x2h{j}", name=f"x2h{j}")
              for j in range(DT)]
    gT = [gp.tile([P, N], bf16, tag=f"g{m}", name=f"g{m}")
          for m in range(FT)]
    expp = pl.open("expp", bufs=2)
    smp = pl.open("smallp", bufs=2)
    drp = pl.open("dramp", bufs=4, space="DRAM")
    wop = pl.open("wo", bufs=1)
    w1p = pl.open("w1p", bufs=3)
    xrp = pl.open("xresid", bufs=2)
    wo_sb = [wop.tile([P, D], bf16, tag=f"wo{j}", name=f"wo{j}")
             for j in range(DT)]
    for j in range(DT):
        nc.sync.dma_start(out=wo_sb[j], in_=wod[ts(j, P), :])

    for c in range(NC_CHUNKS):
        # --- attention chunk c, all head pairs ---
        for hp in range(H // 2):
            h0, h1 = 2 * hp, 2 * hp + 1
            cps0 = ps_main.tile([P, CW], f32, tag="c0", bufs=1, name="c0")
            cps1 = ps_main.tile([P, CW], f32, tag="c1", bufs=1, name="c1")
            for j in range(NT):
                sps = ps_main.tile([P, 2, CW], f32, tag="s", bufs=2,
                                   name="sps")
                nc.tensor.matmul(sps[:, 0, :], kT[hp][0:DH, ts(j, P)],
                                 qT[hp][0:DH, ts(c, CW)],
                                 start=True, stop=True)
                nc.tensor.matmul(sps[:, 1, :], kT[hp][DH:P, ts(j, P)],
                                 qT[hp][DH:P, ts(c, CW)],
                                 start=True, stop=True)
                ee = expp.tile([P, 2, CW], bf16, tag="e", name="ee")
                nc.scalar.activation(out=ee, in_=sps, func=AF.Exp)
                first, last = j == 0, j == NT - 1
                # M=65: col 64 of v3 is ones -> row 64 = softmax row-sum
                nc.tensor.matmul(cps0[0:DH + 1, :], v3[j][:, h0, 0:DH + 1],
                                 ee[:, 0, :], start=first, stop=last)
                nc.tensor.matmul(cps1[0:DH + 1, :], v3[j][:, h1, 0:DH + 1],
                                 ee[:, 1, :], start=first, stop=last)
            for hh, cps in ((h0, cps0), (h1, cps1)):
                # Drain PSUM to f32 SBUF immediately (frees the bank), then
                # normalize off-PSUM.
                cu = smp.tile([DH + 1, CW], f32, tag=f"cu{hh % 2}",
                              name="cu")
                nc.vector.tensor_copy(out=cu, in_=cps[0:DH + 1, :])
                rb = smp.tile([DH + 1, CW], f32, tag=f"rb{hh % 2}",
                              name="rb")
                nc.vector.reciprocal(out=rb[DH:DH + 1, :],
                                     in_=cu[DH:DH + 1, :])
                drow = drp.tile([1, CW], f32, tag=f"drow{hh % 2}",
                                name="drow")
                nc.sync.dma_start(out=drow, in_=rb[DH:DH + 1, :])
                nc.sync.dma_start(out=rb[0:DH, :],
                                  in_=_bcast_ap(drow[0:1, :], DH))
                if hh % 2 == 0:
                    nc.vector.tensor_mul(ctxT[hp][0:DH, ts(c, CW)],
                                         cu[0:DH, :], rb[0:DH, :])
                else:
                    tmp = smp.tile([DH, CW], bf16, tag="tmp", name="tmp")
                    nc.vector.tensor_mul(tmp, cu[0:DH, :], rb[0:DH, :])
                    nc.sync.dma_start(out=ctxT[hp][DH:P, ts(c, CW)],
                                      in_=tmp)

        # --- Wo + residual + LN2 for this chunk's token tiles ---
        for t in range(4 * c, 4 * (c + 1)):
            xr = xrp.tile([P, D], f32, tag="xr", name="xr")
            nc.sync.dma_start(out=xr, in_=xd[ts(t, P), :])
            for lo, w in ((0, 512), (512, 256)):
                ps = ps_main.tile([P, 512], f32, tag="mm", bufs=2,
                                  name="ops")
                for j in range(DT):
                    nc.tensor.matmul(ps[:, 0:w], ctxT[j][:, ts(t, P)],
                                     wo_sb[j][:, lo:lo + w],
                                     start=(j == 0), stop=(j == DT - 1))
                nc.vector.scalar_tensor_tensor(
                    out=x2_sb[t][:, lo:lo + w], in0=ps[:, 0:w], scalar=1.0,
                    in1=xr[:, lo:lo + w], op0=ALU.mult, op1=ALU.add)
            nc.gpsimd.tensor_add(out=x2_sb[t], in0=x2_sb[t], in1=bo_b)
            _ln_tile(nc, lnpools, x2_sb[t], x2hatT, magic_t, ident, t, "l2")
            nc.gpsimd.tensor_add(out=x2_sb[t], in0=x2_sb[t], in1=b2_b)

    # ---------- fc1 + GELU (after attention: single Gelu table load) -----
    for m in range(FT):
        w1m = w1p.tile([P, DT, P], bf16, tag="w1m", name="w1m")
        nc.sync.dma_start(
            out=w1m,
            in_=w1d[:, ts(m, P)].rearrange("(jt p) f -> p jt f", p=P))
        for c in range(NC_CHUNKS):
            ps = ps_main.tile([P, CW], f32, tag="mm", bufs=2, name="f1")
            for j in range(DT):
                nc.tensor.matmul(ps, w1m[:, j, :], x2hatT[j][:, ts(c, CW)],
                                 start=(j == 0), stop=(j == DT - 1))
            nc.scalar.activation(out=gT[m][:, ts(c, CW)], in_=ps,
                                 func=AF.Gelu, bias=b1_sb[:, m:m + 1])

    _tap(nc, "ctx", ctxT)
    _tap(nc, "x2", x2_sb)
    _tap(nc, "x2h", x2hatT)
    _tap(nc, "g", gT)
    pl.close("xresid", "w1p", "wo", "dramp", "smallp", "expp", "qkv",
             "x2hatT")

    # ---------- Phase 6: fc2 + final residual ----------
    w2p = pl.open("w2p", bufs=1)
    outp = pl.open("outp", bufs=3)
    w2_sb = [w2p.tile([P, D], bf16, tag=f"w2_{m}", name=f"w2_{m}")
             for m in range(FT)]
    for m in range(FT):
        nc.sync.dma_start(out=w2_sb[m], in_=w2d[ts(m, P), :])
    for t in range(NT):
        ot = outp.tile([P, D], f32, tag="out", name="ot")
        for lo, w in ((0, 512), (512, 256)):
            ps = ps_main.tile([P, 512], f32, tag="mm", bufs=2, name="f2")
            for m in range(FT):
                nc.tensor.matmul(ps[:, 0:w], gT[m][:, ts(t, P)],
                                 w2_sb[m][:, lo:lo + w],
                                 start=(m == 0), stop=(m == FT - 1))
            nc.vector.scalar_tensor_tensor(
                out=ot[:, lo:lo + w], in0=ps[:, 0:w], scalar=1.0,
                in1=x2_sb[t][:, lo:lo + w], op0=ALU.mult, op1=ALU.add)
        nc.sync.dma_start(out=outd[ts(t, P), :], in_=ot)


def _get_program():
    global _PROGRAM
    if _PROGRAM is None:
        _PROGRAM = _build_program()
    return _PROGRAM


def _prepare_host_inputs(inputs):
    f64 = np.float64
    x = np.asarray(inputs["x"], np.float32)
    g1 = np.asarray(inputs["ln1_g"], f64)
    b1l = np.asarray(inputs["ln1_b"], f64)
    g2 = np.asarray(inputs["ln2_g"], f64)
    b2l = np.asarray(inputs["ln2_b"], f64)
    Wq = np.asarray(inputs["Wq"], f64)
    Wk = np.asarray(inputs["Wk"], f64)
    Wv = np.asarray(inputs["Wv"], f64)
    Wo = np.asarray(inputs["Wo"], f64)
    W1 = np.asarray(inputs["W1"], f64)
    W2 = np.asarray(inputs["W2"], f64)
    bq = np.asarray(inputs["bq"], f64)
    bk = np.asarray(inputs["bk"], f64)
    bv = np.asarray(inputs["bv"], f64)
    bo = np.asarray(inputs["bo"], f64)
    b1 = np.asarray(inputs["b1"], f64)
    b2 = np.asarray(inputs["b2"], f64)

    def bf(a):
        return np.ascontiguousarray(a.astype(np.float32)).astype(
            ml_dtypes.bfloat16)

    def col_tile(vec, nt):  # [nt*P] -> [P, nt]
        return np.ascontiguousarray(vec.astype(np.float32).reshape(nt, P).T)

    wq_h = bf(g1[:, None] * Wq * 0.125)
    bq_h = col_tile((b1l @ Wq + bq) * 0.125, DT)
    wk_h = bf(g1[:, None] * Wk)
    bk_h = col_tile(b1l @ Wk + bk, DT)
    wv_h = bf(g1[:, None] * Wv)
    bv_f = b1l @ Wv + bv
    wo_h = bf(Wo)
    bo_f = bo + bv_f @ Wo
    bob_h = np.ascontiguousarray(
        np.broadcast_to(bo_f.astype(np.float32), (P, D)))
    w1_h = bf(g2[:, None] * W1)
    b1_h = col_tile(b2l @ W1 + b1, FT)
    w2_h = bf(W2)
    b2b_h = np.ascontiguousarray(
        np.broadcast_to(b2.astype(np.float32), (P, D)))

    shared = {"wq": wq_h, "wk": wk_h, "wv": wv_h, "wo": wo_h,
              "w1": w1_h, "w2": w2_h, "bq": bq_h, "bk": bk_h,
              "b1": b1_h, "bob": bob_h, "b2b": b2b_h}
    return x, shared


def kernel(**inputs):
    x, shared = _prepare_host_inputs(inputs)
    nc = _get_program()
    in_maps = [dict(shared, x=np.ascontiguousarray(x[c]))
               for c in range(NCORES)]
    import time
    t0 = time.perf_counter()
    res = run_bass_kernel_spmd(nc, in_maps, list(range(NCORES)))
    t1 = time.perf_counter()
    kernel._last_wall_s = t1 - t0
    out = np.stack([res.results[c]["out"] for c in range(NCORES)], axis=0)
    return out.astype(np.float32)
